# revision 1
# baseline (speedup 1.0000x reference)
"""ArcFace loss (mean softmax-CE over 100k classes) on 8 TRN2 NeuronCores.

Strategy: classification/tensor parallel — shard the class axis (100000)
across 8 cores (12500 each, zero-padded to 12800). Each core streams its
normalized-transposed weight shard [384, 12800] (bf16) from HBM, computes
cos = emb @ W_hat.T via TensorEngine (bf16, K=384, N=512 tiles), then
exp(64*cos) on ScalarE (PSUM -> SBUF bf16) and a VectorE chain
(min-cap at 4x perf mode, a fused min+add fold halving the class dim,
and a 1x tensor_reduce) producing a per-shard sum-of-exp per row.

Key math simplification: logits are bounded in [-64.2, 64] (cos clipped to
+-1, S=64), and exp(64) ~ 6.2e27 fits in fp32 — so NO max-subtraction and
no cross-core max reduction is needed. The clip(cos, -1+eps, 1-eps) before
exp is equivalent to min(exp(64*cos), exp(64*(1-eps))) after exp (exp is
monotonic; the lower clip only matters at the 1e-28 level, far below the
sum's fp32 resolution).

The label column (ArcFace margin) affects exactly one class per row; the
host swaps the plain-cos exp term for the margin term using an exact fp64
computation (512 dot products — negligible), then finishes:
    nll_b = log(sumexp_b) - 64*phi_b;  out = mean(nll).
"""

import math
import os
import sys

for _p in ("/opt/trn_rl_repo",):
    if os.path.isdir(_p) and _p not in sys.path:
        sys.path.insert(0, _p)

import numpy as np
import ml_dtypes

import concourse.bass as bass
import concourse.mybir as mybir
import concourse.tile as tile
from concourse.bass_utils import run_bass_kernel_spmd
from concourse.tile_rust import add_dep_helper

NUM_CLASSES = 100000
EMBED = 384
BATCH = 512
S = 64.0
M = 0.5
COS_M = math.cos(M)
SIN_M = math.sin(M)
TH = math.cos(math.pi - M)
MM = math.sin(math.pi - M) * M
EPS = 1e-07

N_CORES = 8
C_SHARD = NUM_CLASSES // N_CORES          # 12500
C_TILE = 512
N_TILES = (C_SHARD + C_TILE - 1) // C_TILE  # 25
C_PAD = N_TILES * C_TILE                  # 12800
K_CHUNKS = EMBED // 128                   # 3
B_BLOCKS = BATCH // 128                   # 4
CAP = float(np.exp(np.float64(S * (1.0 - EPS))))  # exp(63.9999936)

_cache: dict = {}


def _build_nc(split_waits: bool = True) -> bass.Bass:
    # target_bir_lowering=True: assemble BIR for walrus's BIR-lowering
    # pipeline (the path bass2jax/axon compiles through).
    nc = bass.Bass(target_bir_lowering=True)
    wt = nc.declare_dram_parameter(
        "wt", [N_TILES, 128, K_CHUNKS, C_TILE], mybir.dt.bfloat16, isOutput=False
    )
    embt = nc.declare_dram_parameter(
        "embt", [128, K_CHUNKS, BATCH], mybir.dt.bfloat16, isOutput=False
    )
    out = nc.declare_dram_parameter(
        "out", [128, B_BLOCKS], mybir.dt.float32, isOutput=True
    )

    with tile.TileContext(nc) as tc:
        with (
            tc.tile_pool(name="wtp", bufs=N_TILES) as wt_pool,
            tc.tile_pool(name="expp", bufs=N_TILES) as exp_pool,
            tc.tile_pool(name="junk", bufs=N_TILES) as junk_pool,
            tc.tile_pool(name="small", bufs=1) as small,
            tc.tile_pool(name="psum", bufs=2, space="PSUM") as psum_pool,
        ):
            embt_s = small.tile([128, K_CHUNKS, BATCH], mybir.dt.bfloat16)
            nc.gpsimd.dma_start(out=embt_s[:], in_=embt[:])
            # Per-tile partial sums kept in bf16 so the DVE reduce runs in
            # its fast (2x/4x) perf mode; the reduce's internal accumulator
            # is fp32, only the 512-term partial store rounds to bf16
            # (~0.4% noise on a partial -> ~1e-5 on the final mean loss).
            sums = small.tile([128, N_TILES, B_BLOCKS], mybir.dt.bfloat16)
            outs = small.tile([128, B_BLOCKS], mybir.dt.float32)

            # PE warm-up: the HAM clock gate holds the PE at 1.2GHz until
            # ~3.4us of sustained activity. The first real matmul can't
            # start until the first weight tile lands (~14us in), so burn
            # the DMA-wait window with dep-free dummy matmuls on garbage
            # SBUF (never read back; each real accumulation group opens
            # with start=True, which resets the bank).
            jw = small.tile([128, 640], mybir.dt.bfloat16)
            nc.vector.memset(jw[:], 0.0)
            warm = psum_pool.tile(
                [128, B_BLOCKS, C_TILE], mybir.dt.float32, tag="ps"
            )
            # 20 dummies (~4.3us) span the whole DMA-wait window: the gap
            # between the last dummy and the first real matmul must stay
            # under the ~3.4us HAM MID window or the PE re-throttles.
            for i in range(20):
                nc.tensor.matmul(
                    warm[:, i % B_BLOCKS, :],
                    jw[:, 512:640],
                    jw[:, 0:512],
                    start=True,
                    stop=True,
                )

            acts = []
            for ct in range(N_TILES):
                wt_t = wt_pool.tile([128, K_CHUNKS, C_TILE], mybir.dt.bfloat16)
                nc.gpsimd.dma_start(out=wt_t[:], in_=wt[ct])

                # Walrus allows only ONE sync wait per instruction on this
                # lowering path. The first matmul of a tile group would need
                # several (wt-DMA lane, psum-slot release from ACT, prior PE
                # group completion). Spare LDWEIGHTS ops absorb all but the
                # own-engine wait so PE observes those ticks first.
                nc.tensor.ldweights(wt_t[:, 0, 0:1])  # absorbs wt-DMA wait
                if ct >= 2:
                    ldw = nc.tensor.ldweights(embt_s[:, 0, 0:1])
                    add_dep_helper(
                        ldw.ins,
                        acts[ct - 2].ins,
                        sync=True,
                        reason="absorb psum-release wait on PE",
                    )

                ps = psum_pool.tile([128, B_BLOCKS, C_TILE], mybir.dt.float32)
                for b in range(B_BLOCKS):
                    for k in range(K_CHUNKS):
                        nc.tensor.matmul(
                            ps[:, b, :],
                            embt_s[:, k, b * 128 : (b + 1) * 128],
                            wt_t[:, k, :],
                            start=(k == 0),
                            stop=(k == K_CHUNKS - 1),
                        )

                et = exp_pool.tile([128, B_BLOCKS, C_TILE], mybir.dt.bfloat16)
                act = nc.scalar.activation(
                    out=et[:],
                    in_=ps[:],
                    func=mybir.ActivationFunctionType.Exp,
                    scale=S,
                )
                acts.append(act)
                # DVE chain. tensor_reduce has NO fast perf mode (1 elem/
                # lane/cycle @0.96GHz), so halve its input first using ops
                # that DO: tensor_scalar min runs 4x, scalar_tensor_tensor
                # (min+add fused) runs 2x. bf16 pair-sums only add
                # ~0.4%/sqrt(n) noise to a 256-term partial.
                nc.vector.tensor_scalar(
                    out=et[:, :, 256:512],
                    in0=et[:, :, 256:512],
                    scalar1=CAP,
                    scalar2=None,
                    op0=mybir.AluOpType.min,
                )
                nc.vector.scalar_tensor_tensor(
                    out=et[:, :, 0:256],
                    in0=et[:, :, 0:256],
                    scalar=CAP,
                    in1=et[:, :, 256:512],
                    op0=mybir.AluOpType.min,
                    op1=mybir.AluOpType.add,
                )
                with nc.allow_low_precision(
                    "bf16 per-tile partials; fp32 internal accumulator"
                ):
                    nc.vector.tensor_reduce(
                        out=sums[:, ct, :],
                        in_=et[:, :, 0:256],
                        axis=mybir.AxisListType.X,
                        op=mybir.AluOpType.add,
                    )

            for b in range(B_BLOCKS):
                nc.vector.tensor_reduce(
                    out=outs[:, b : b + 1],
                    in_=sums[:, :, b],
                    axis=mybir.AxisListType.X,
                    op=mybir.AluOpType.add,
                )
            nc.sync.dma_start(out=out[:], in_=outs[:])

    if split_waits:
        _split_multi_waits(nc)
    return nc


def _split_multi_waits(nc: bass.Bass) -> None:
    """This walrus build accepts only ONE sync wait per instruction. Tile's
    kernel-tail drain waits on every proc's final tick (~12 waits). Split any
    multi-wait instruction into a ladder of same-engine NOPs, one wait each,
    inserted immediately before it (sequential waits on one sequencer are a
    logical AND, so semantics are unchanged)."""
    for f in nc.m.functions:
        for bb in f.blocks:
            insts = list(bb.instructions)
            if not any(
                ins.sync_info is not None
                and ins.sync_info.on_wait
                and len(ins.sync_info.on_wait) > 1
                for ins in insts
            ):
                continue
            new_insts = []
            for ins in insts:
                si = ins.sync_info
                if si is not None and si.on_wait and len(si.on_wait) > 1:
                    waits = list(si.on_wait)
                    for j, w in enumerate(waits[:-1]):
                        nop = mybir.InstEventSemaphore(
                            name=f"{ins.name}-waitsplit-{j}",
                            ins=[],
                            outs=[],
                        )
                        nop.engine = ins.engine
                        nop.sync_info = mybir.SyncInfo(on_wait=[w], on_update=[])
                        new_insts.append(nop)
                    ins.sync_info = mybir.SyncInfo(
                        on_wait=[waits[-1]], on_update=list(si.on_update or [])
                    )
                new_insts.append(ins)
            bb.instructions = new_insts


def _get_nc() -> bass.Bass:
    if "nc" not in _cache:
        _cache["nc"] = _build_nc()
    return _cache["nc"]


def _make_in_maps(embeddings: np.ndarray, weight: np.ndarray):
    w = np.asarray(weight, dtype=np.float32)
    norms = np.sqrt(np.einsum("ce,ce->c", w, w, dtype=np.float64))
    wn = w / norms[:, None].astype(np.float32)  # [C, E] f32, rows unit-norm

    wn_pad = np.zeros((N_CORES, C_PAD, EMBED), np.float32)
    wn_pad[:, :C_SHARD, :] = wn.reshape(N_CORES, C_SHARD, EMBED)
    # [core, tile, j, k, p] -> [core, tile, p, k, j]
    wt_all = np.ascontiguousarray(
        wn_pad.reshape(N_CORES, N_TILES, C_TILE, K_CHUNKS, 128).transpose(0, 1, 4, 3, 2)
    ).astype(ml_dtypes.bfloat16)

    emb = np.asarray(embeddings, dtype=np.float32)
    embt = np.ascontiguousarray(
        emb.T.reshape(K_CHUNKS, 128, BATCH).transpose(1, 0, 2)
    ).astype(ml_dtypes.bfloat16)

    in_maps = [{"wt": wt_all[c], "embt": embt} for c in range(N_CORES)]
    return in_maps, norms


def _host_finish(embeddings, labels, weight, norms, sumexp):
    """Exact fp64 label-term swap + final log/mean."""
    emb = np.asarray(embeddings, dtype=np.float64)
    lab = np.asarray(labels).astype(np.int64)
    w = np.asarray(weight, dtype=np.float64)

    wl = w[lab] / norms[lab][:, None]              # [B, E] unit rows
    cos_l = np.einsum("be,be->b", emb, wl)         # true label cos (unclipped)
    c = np.clip(cos_l, -1.0 + EPS, 1.0 - EPS)
    sin_l = np.sqrt(1.0 - c * c)
    cos_m = c * COS_M - sin_l * SIN_M
    phi = np.where(c > TH, cos_m, c - MM)

    # device's plain-cos contribution for the label column (top-capped only)
    t_plain = np.exp(S * np.minimum(cos_l, 1.0 - EPS))
    t_mod = np.exp(S * phi)

    total = sumexp - t_plain + t_mod
    nll = np.log(total) - S * phi
    return np.asarray(np.mean(nll), dtype=np.float32)


def _run_device(in_maps, trace=False, **kw):
    nc = _get_nc()
    return run_bass_kernel_spmd(nc, in_maps, core_ids=list(range(N_CORES)),
                                trace=trace, **kw)


def kernel(embeddings: np.ndarray, labels: np.ndarray, weight: np.ndarray) -> np.ndarray:
    in_maps, norms = _make_in_maps(embeddings, weight)
    res = _run_device(in_maps)
    # per-core out: [128, B_BLOCKS] f32; row b = blk*128 + p  ->  out[p, blk]
    sumexp = np.zeros(BATCH, np.float64)
    for r in res.results:
        sumexp += r["out"].astype(np.float64).T.reshape(BATCH)
    return _host_finish(embeddings, labels, weight, norms, sumexp)

